# revision 3
# baseline (speedup 1.0000x reference)
"""Trainium2 Bass kernel for nn_MultiHeadAttention_5334349382389 (v2).

Sharding: 8 cores = 4 batches x 2 head-groups (4 heads each).
Core c handles batch b = c // 2, head-group g = c % 2 (heads 4g..4g+3).

Per-core math (fp16 matmuls, fp32 PSUM accumulate):
  qhT = (Wq_g/8) @ x_b^T + bq_g/8        [256, 1024]   (score scale folded into Wq)
  khT = Wk_g @ x_b^T + bk_g              [256, 1024]
  vh  = x_b @ Wv_g^T                     [1024, 256]   (bv folded into host-side bias)
  per head h: scoresT[k,q]; h==0 adds edgeT on the DVE in front of the exp
      (edgeT is zeros on non-edge cores; Wq/bq head-0 slice zeroed on edge
      cores, so edge cores get scoresT == edgeT exactly)
  expT = exp(scoresT)                    (no max-subtraction; inputs bounded)
  outT_raw[d,q] accum over k-tiles with lhsT = [vh | ones] -> row 64 = denom
  OT = outT_raw[:64] * bcast(1/denom)
  partial = OT^T-contraction @ WoT_g     [1024, 512]
Host: out[b] = partial(b,0) + partial(b,1) + (bo + Wo @ bv).

v2 restructure vs v1 (104us -> target ~60us):
- fine-grained per-chunk input DMAs on three engine queues (sync: q-stream,
  scalar: k-stream, gpsimd: v-stream) so the projections chase the DMAs
  instead of waiting on monolithic packed loads (~14us startup stall)
- PE clock-ramp junk matmuls run on a memset tile (no DMA dependency)
- edge add moved off the PE (was I@edgeT matmuls) onto the DVE as a fused
  psum+sbuf->sbuf add feeding the exp
- v projection interleaved into head 0's score loop (chases the v DMA tail)
- PV matmuls software-pipelined one k-tile behind the score matmuls so the
  PE never waits on the Act-engine exp
- per-m-tile output stores issued as soon as each out-proj tile is copied
"""

import os
import sys

sys.path.insert(0, "/opt/trn_rl_repo")

import numpy as np

B, SEQ, DIN, DO = 4, 1024, 512, 512
NH_ALL, DK = 8, 64
NHC = 4            # heads per core
DH = NHC * DK      # 256 per-core projected dims
P = 128
CD = DIN // P      # 4 contraction chunks for projections
CH = DH // P       # 2 dh chunks
KT = SEQ // P      # 8 k-tiles
STR = 512          # q-stripe (matmul free dim)
NS = SEQ // STR    # 2 stripes
TVW = NHC * (DK + 1) + DK - 1  # 323: per-k-tile aux width (4x65 + 63 pad)

NJUNK0 = 10        # initial clock-ramp junk matmuls
NJUNK_CHASE = 2    # junk per proj cd-chunk while chasing DMA
NJUNK_PRE = 4      # junk right before head 0

COMPUTE = os.environ.get("KERNEL_COMPUTE_DT", "fp16")  # fp16 | bf16 | fp32r

_nc = None


def _np_dt():
    import ml_dtypes

    return {
        "fp16": np.float16,
        "bf16": ml_dtypes.bfloat16,
        "fp32r": np.float32,
    }[COMPUTE]


def _build():
    global _nc
    if _nc is not None:
        return _nc
    import concourse.bacc as bacc
    import concourse.bass as bass
    import concourse.mybir as mybir
    import concourse.tile as tile

    f32 = mybir.dt.float32
    f32r = mybir.dt.float32r
    cdt = {
        "fp16": mybir.dt.float16,
        "bf16": mybir.dt.bfloat16,
        "fp32r": f32r,
    }[COMPUTE]
    Exp = mybir.ActivationFunctionType.Exp

    nc = bacc.Bacc("TRN2", target_bir_lowering=False, debug=False)

    wq_d = nc.dram_tensor("wq", (P, CD * DH), cdt, kind="ExternalInput")
    wk_d = nc.dram_tensor("wk", (P, CD * DH), cdt, kind="ExternalInput")
    wv_d = nc.dram_tensor("wv", (P, CD * DH), cdt, kind="ExternalInput")
    wo_d = nc.dram_tensor("wo", (P, CH * DO), cdt, kind="ExternalInput")
    xq_d = nc.dram_tensor("xq", (P, CD * SEQ), cdt, kind="ExternalInput")
    xk_d = nc.dram_tensor("xk", (P, CD * SEQ), cdt, kind="ExternalInput")
    xv_d = nc.dram_tensor("xv", (P, KT * CD * P), cdt, kind="ExternalInput")
    bqk = nc.dram_tensor("bqk", (2 * DH, 1), f32, kind="ExternalInput")
    edge = nc.dram_tensor("edge", (SEQ, SEQ), cdt, kind="ExternalInput")
    outp = nc.dram_tensor("outp", (SEQ, DO), cdt, kind="ExternalOutput")

    edge_r = edge.rearrange("(t p) n -> t p n", p=P)
    xq_r = xq_d.rearrange("p (c n) -> p c n", n=SEQ)
    xk_r = xk_d.rearrange("p (c n) -> p c n", n=SEQ)
    xv_r = xv_d.rearrange("p (t w) -> p t w", w=CD * P)
    out_r = outp.rearrange("(t p) n -> p t n", p=P)

    def sl(s):
        return slice(s * STR, (s + 1) * STR)

    with tile.TileContext(nc) as tc:
        with (
            tc.tile_pool(name="inp", bufs=1) as inp,
            tc.tile_pool(name="wts", bufs=1) as wts,
            tc.tile_pool(name="qkp", bufs=1) as qkp,
            tc.tile_pool(name="vhap", bufs=1) as vhap,
            tc.tile_pool(name="expp", bufs=6) as expp,
            tc.tile_pool(name="teip", bufs=3) as teip,
            tc.tile_pool(name="otp", bufs=1) as otp,
            tc.tile_pool(name="rrp", bufs=4) as rrp,
            tc.tile_pool(name="rbp", bufs=4) as rbp,
            tc.tile_pool(name="oalp", bufs=3) as oalp,
            tc.tile_pool(name="edgp", bufs=4) as edgp,
            tc.tile_pool(name="bigp", bufs=2, space=bass.MemorySpace.PSUM) as bigp,
            tc.tile_pool(name="pvp", bufs=2, space=bass.MemorySpace.PSUM) as pvp,
            tc.tile_pool(name="vpp", bufs=1, space=bass.MemorySpace.PSUM) as vpp,
            tc.tile_pool(name="jnkp", bufs=1, space=bass.MemorySpace.PSUM) as jnkp,
        ):
            # ---------------- tiles ----------------
            tjk = wts.tile([P, STR], cdt, tag="tjk")
            twq = wts.tile([P, CD, DH], cdt, tag="twq")
            twk = wts.tile([P, CD, DH], cdt, tag="twk")
            twv = wts.tile([P, CD, DH], cdt, tag="twv")
            two = wts.tile([P, CH, DO], cdt, tag="two")
            tb4 = wts.tile([P, 4, 1], f32, tag="tb4")
            txq = inp.tile([P, CD, SEQ], cdt, tag="txq")
            txk = inp.tile([P, CD, SEQ], cdt, tag="txk")
            txv = inp.tile([P, KT, CD, P], cdt, tag="txv")
            tqh = qkp.tile([P, CH, SEQ], cdt, tag="tqh")
            khp = qkp.tile([P, NHC, SEQ], cdt, tag="khp")
            tvha = vhap.tile([P, KT, TVW], cdt, tag="tvha")
            tot = otp.tile([P, CH, SEQ], cdt, tag="tot")

            # ------- gpsimd queue: junk tile, v-stream DMAs, memsets -------
            nc.gpsimd.memset(tjk, 0.0)
            nc.gpsimd.dma_start(
                out=twv, in_=wv_d.rearrange("p (c d) -> p c d", d=DH)
            )
            for st in range(KT):
                nc.gpsimd.dma_start(out=txv[:, st], in_=xv_r[:, st])
            nc.gpsimd.dma_start(
                out=two, in_=wo_d.rearrange("p (c d) -> p c d", d=DO)
            )
            # zero the unused partition-halves of khp (even heads: parts
            # 64-127, odd heads: parts 0-63) so score matmuls see zero weights
            nc.gpsimd.memset(khp[0:DK, 1::2, :], 0.0)
            nc.gpsimd.memset(khp[DK:P, 0::2, :], 0.0)
            # vh-aug tail pad + per-head ones columns (denominator rows)
            nc.gpsimd.memset(tvha[:, :, NHC * (DK + 1) : TVW], 0.0)
            nc.gpsimd.memset(
                tvha[:, :, 0 : NHC * (DK + 1)].rearrange(
                    "p t (h w) -> p t h w", w=DK + 1
                )[:, :, :, DK : DK + 1],
                1.0,
            )

            # ------- sync queue: q-stream DMAs (edge tiles issued in head 0)
            nc.sync.dma_start(out=tb4, in_=bqk.rearrange("(c p) o -> p c o", p=P))
            nc.sync.dma_start(out=twq, in_=wq_d.rearrange("p (c d) -> p c d", d=DH))
            for cd in range(CD):
                nc.sync.dma_start(out=txq[:, cd], in_=xq_r[:, cd])

            # ------- scalar queue: k-stream DMAs -------
            nc.scalar.dma_start(out=twk, in_=wk_d.rearrange("p (c d) -> p c d", d=DH))
            for cd in range(CD):
                nc.scalar.dma_start(out=txk[:, cd], in_=xk_r[:, cd])

            # PE clock-ramp filler: junk matmuls on the memset tile keep the
            # p-state ramp going while DMAs land. Dedicated PSUM bank, never
            # read.
            jnk = jnkp.tile([P, STR], f32, tag="jnk")

            def junk(n):
                for _ in range(n):
                    nc.tensor.matmul(
                        jnk[:], lhsT=tjk[:, 0:P], rhs=tjk[:], start=True, stop=True
                    )

            junk(NJUNK0)

            # ---------------- q/k projections ----------------
            def proj_q(ch, jn=0):
                pt = bigp.tile([P, SEQ], f32, tag="big")
                for cd in range(CD):
                    for s in range(NS):
                        nc.tensor.matmul(
                            pt[:, sl(s)],
                            lhsT=twq[:, cd, ch * P : (ch + 1) * P],
                            rhs=txq[:, cd, sl(s)],
                            start=(cd == 0),
                            stop=(cd == CD - 1),
                        )
                    if jn:
                        junk(jn)
                nc.vector.tensor_scalar_add(
                    out=tqh[:, ch, :], in0=pt[:], scalar1=tb4[:, ch, :]
                )

            def proj_k(ch, jn=0):
                pt = bigp.tile([P, SEQ], f32, tag="big")
                for cd in range(CD):
                    for s in range(NS):
                        nc.tensor.matmul(
                            pt[:, sl(s)],
                            lhsT=twk[:, cd, ch * P : (ch + 1) * P],
                            rhs=txk[:, cd, sl(s)],
                            start=(cd == 0),
                            stop=(cd == CD - 1),
                        )
                    if jn:
                        junk(jn)
                nc.vector.tensor_scalar_add(
                    out=khp[0:DK, 2 * ch, :],
                    in0=pt[0:DK, :],
                    scalar1=tb4[0:DK, 2 + ch, :],
                )
                nc.vector.tensor_scalar_add(
                    out=khp[DK:P, 2 * ch + 1, :],
                    in0=pt[DK:P, :],
                    scalar1=tb4[DK:P, 2 + ch, :],
                )

            proj_q(0, jn=NJUNK_CHASE)
            proj_k(0, jn=NJUNK_CHASE)
            junk(NJUNK_PRE)

            # ---------------- attention per head ----------------
            def head_body(h):
                ch, off = h // 2, (h % 2) * DK
                first = h == 0
                pv0 = pvp.tile([P, STR], f32, tag="pv")
                pv1 = pvp.tile([P, STR], f32, tag="pv")
                pvs = (pv0, pv1)

                def pv_mm(lte, lkt, stop):
                    for s in range(NS):
                        nc.tensor.matmul(
                            pvs[s][:],
                            lhsT=tvha[:, lkt, h * (DK + 1) : h * (DK + 1) + P],
                            rhs=lte[:, sl(s)],
                            start=(lkt == 0),
                            stop=stop,
                        )

                lag = None
                for kt in range(KT):
                    if first:
                        ed = edgp.tile([P, SEQ], cdt, tag="edg")
                        nc.sync.dma_start(out=ed, in_=edge_r[kt])
                    stt = bigp.tile([P, SEQ], f32, tag="big")
                    for s in range(NS):
                        nc.tensor.matmul(
                            stt[:, sl(s)],
                            lhsT=khp[:, h, kt * P : (kt + 1) * P],
                            rhs=tqh[:, ch, sl(s)],
                            start=True,
                            stop=True,
                        )
                    if first:
                        # v projection for k-tile kt, chasing the v DMA tail
                        vp = vpp.tile([P, STR], f32, tag="vp")
                        for cd in range(CD):
                            nc.tensor.matmul(
                                vp[:, 0:DH],
                                lhsT=txv[:, kt, cd, :],
                                rhs=twv[:, cd, :],
                                start=(cd == 0),
                                stop=(cd == CD - 1),
                            )
                    te = expp.tile([P, SEQ], cdt, tag="expT")
                    if first:
                        nc.vector.tensor_copy(
                            out=tvha[:, kt, 0 : NHC * (DK + 1)].rearrange(
                                "p (h w) -> p h w", w=DK + 1
                            )[:, :, 0:DK],
                            in_=vp[:, 0:DH].rearrange("p (h d) -> p h d", h=NHC),
                        )
                        tei = teip.tile([P, SEQ], cdt, tag="tein")
                        nc.vector.tensor_add(out=tei[:], in0=stt[:], in1=ed[:])
                        nc.scalar.activation(out=te, in_=tei[:], func=Exp)
                    else:
                        nc.scalar.activation(out=te, in_=stt[:], func=Exp)
                    if lag is not None:
                        pv_mm(lag[0], lag[1], stop=False)
                    lag = (te, kt)
                pv_mm(lag[0], lag[1], stop=True)

                for s in range(NS):
                    rr = rrp.tile([1, STR], f32, tag="rr")
                    rs = rrp.tile([1, STR], f32, tag="rs")
                    nc.vector.tensor_copy(out=rs[:], in_=pvs[s][DK : DK + 1, :])
                    nc.vector.reciprocal_approx_fast(out=rr[:], in_=rs[:])
                    rb = rbp.tile([DK, STR], f32, tag="rb")
                    nc.gpsimd.partition_broadcast(rb[:], rr[:])
                    nc.vector.tensor_mul(
                        tot[off : off + DK, ch, sl(s)], pvs[s][0:DK, :], rb[:]
                    )

            head_body(0)
            head_body(1)
            proj_q(1)
            proj_k(1)
            head_body(2)
            head_body(3)

            # ---------------- output projection ----------------
            store_eng = [
                nc.sync, nc.gpsimd, nc.sync, nc.gpsimd,
                nc.sync, nc.gpsimd, nc.scalar, nc.scalar,
            ]
            for m in range(KT):
                po = bigp.tile([P, SEQ], f32, tag="big")
                for ci in range(CH):
                    nc.tensor.matmul(
                        po[:, 0:DO],
                        lhsT=tot[:, ci, m * P : (m + 1) * P],
                        rhs=two[:, ci, :],
                        start=(ci == 0),
                        stop=(ci == CH - 1),
                    )
                oal = oalp.tile([P, DO], cdt, tag="oall")
                nc.vector.tensor_copy(out=oal[:], in_=po[:, 0:DO])
                store_eng[m].dma_start(out=out_r[:, m], in_=oal[:])

    nc.compile()
    _nc = nc
    return nc


def _in_maps(q, k, v, edge_matrix, Wq, bq, Wk, bk, Wv, Wo):
    dt = _np_dt()
    zeros_edge = np.zeros((SEQ, SEQ), dt)
    edge_t = np.ascontiguousarray(edge_matrix.T).astype(dt)

    def re_cp(m):
        # [C*P, D] -> [P, C*D] (partition-major packing of "(c p) d -> p c d")
        cp, d = m.shape
        return np.ascontiguousarray(
            m.reshape(cp // P, P, d).transpose(1, 0, 2).reshape(P, -1)
        )

    def re_st(m):
        # [CD*P, KT*P] -> [P, KT*CD*P]: st-major packing for the v stream
        return np.ascontiguousarray(
            m.reshape(CD, P, KT, P).transpose(1, 2, 0, 3).reshape(P, -1)
        )

    xt = {}
    for b in range(B):
        xt[b] = (
            re_cp(np.ascontiguousarray(q[b].T).astype(dt)),
            re_cp(np.ascontiguousarray(k[b].T).astype(dt)),
            re_st(np.ascontiguousarray(v[b].T).astype(dt)),
        )
    maps = []
    for c in range(8):
        b, g = c // 2, c % 2
        is_edge = g == 0 and b < 2
        rows = slice(g * DH, (g + 1) * DH)
        wq_c = np.ascontiguousarray(Wq[rows].T) * np.float32(1.0 / 8.0)
        bq_c = (bq[rows] * np.float32(1.0 / 8.0)).copy()
        if is_edge:
            wq_c[:, 0:DK] = 0.0
            bq_c[0:DK] = 0.0
        maps.append(
            {
                "wq": re_cp(wq_c.astype(dt)),
                "wk": re_cp(np.ascontiguousarray(Wk[rows].T).astype(dt)),
                "wv": re_cp(np.ascontiguousarray(Wv[rows].T).astype(dt)),
                "wo": re_cp(np.ascontiguousarray(Wo[:, rows].T).astype(dt)),
                "xq": xt[b][0],
                "xk": xt[b][1],
                "xv": xt[b][2],
                "bqk": np.concatenate([bq_c, bk[rows]]).reshape(2 * DH, 1),
                "edge": edge_t if is_edge else zeros_edge,
            }
        )
    return maps


def _ensure_ntff_hook():
    """Register the axon NTFF profile hook if the image's antenv lacks it."""
    import contextlib
    import ctypes
    import types

    try:
        from antenv.axon_hooks import get_axon_ntff_profile_hook  # noqa: F401
        return
    except ImportError:
        pass

    so_path = "/opt/axon/libaxon_pjrt.so"
    try:
        lib = ctypes.CDLL(so_path)
    except OSError:
        return
    if not hasattr(lib, "axon_start_nrt_profile"):
        return
    lib.axon_start_nrt_profile.argtypes = [
        ctypes.POINTER(ctypes.c_int64),
        ctypes.c_size_t,
    ]
    lib.axon_start_nrt_profile.restype = ctypes.c_int64
    lib.axon_stop_nrt_profile.argtypes = [ctypes.c_char_p]
    lib.axon_stop_nrt_profile.restype = ctypes.c_int64

    @contextlib.contextmanager
    def _hook(output_dir, device_ids):
        import jax

        jax.devices()
        if device_ids:
            ids = (ctypes.c_int64 * len(device_ids))(*device_ids)
            rc = lib.axon_start_nrt_profile(ids, len(device_ids))
        else:
            rc = lib.axon_start_nrt_profile(None, 0)
        if rc != 0:
            raise RuntimeError(f"axon_start_nrt_profile rc={rc}")
        try:
            yield
        finally:
            n = lib.axon_stop_nrt_profile(str(output_dir).encode())
            if n < 0:
                raise RuntimeError(f"axon_stop_nrt_profile rc={n}")

    _state = {"hook": _hook}
    mod = types.ModuleType("antenv.axon_hooks")
    mod.get_axon_ntff_profile_hook = lambda: _state["hook"]
    mod.set_axon_ntff_profile_hook = lambda h: _state.__setitem__("hook", h)
    import antenv

    antenv.axon_hooks = mod
    sys.modules["antenv.axon_hooks"] = mod


def kernel(q, k, v, edge_matrix, Wq, bq, Wk, bk, Wv, bv, Wo, bo, _trace=False):
    from concourse.bass_utils import run_bass_kernel_spmd

    if _trace:
        _ensure_ntff_hook()

    q, k, v = (np.asarray(t, np.float32) for t in (q, k, v))
    edge_matrix = np.asarray(edge_matrix, np.float32)
    Wq, bq, Wk, bk, Wv, bv, Wo, bo = (
        np.asarray(t, np.float32) for t in (Wq, bq, Wk, bk, Wv, bv, Wo, bo)
    )

    nc = _build()
    maps = _in_maps(q, k, v, edge_matrix, Wq, bq, Wk, bk, Wv, Wo)
    res = run_bass_kernel_spmd(nc, maps, core_ids=list(range(8)), trace=_trace)

    bo_eff = bo + Wo @ bv
    out = np.empty((B, SEQ, DO), np.float32)
    for b in range(B):
        out[b] = res.results[2 * b]["outp"] + res.results[2 * b + 1]["outp"] + bo_eff
    if _trace:
        return out, res
    return out


# revision 26
# speedup vs baseline: 1.0670x; 1.0670x over previous
"""Trainium2 Bass kernel for nn_MultiHeadAttention_5334349382389 (v2).

Sharding: 8 cores = 4 batches x 2 head-groups (4 heads each).
Core c handles batch b = c // 2, head-group g = c % 2 (heads 4g..4g+3).

Per-core math (fp16 matmuls, fp32 PSUM accumulate):
  qhT = (Wq_g/8) @ x_b^T + bq_g/8        [256, 1024]   (score scale folded into Wq)
  khT = Wk_g @ x_b^T + bk_g              [256, 1024]
  vh  = x_b @ Wv_g^T                     [1024, 256]   (bv folded into host-side bias)
  per head h: scoresT[k,q]; h==0 adds edgeT on the DVE in front of the exp
      (edgeT is zeros on non-edge cores; Wq/bq head-0 slice zeroed on edge
      cores, so edge cores get scoresT == edgeT exactly)
  expT = exp(scoresT)                    (no max-subtraction; inputs bounded)
  outT_raw[d,q] accum over k-tiles with lhsT = [vh | ones] -> row 64 = denom
  OT = outT_raw[:64] * bcast(1/denom)
  partial = OT^T-contraction @ WoT_g     [1024, 512]
Host: out[b] = partial(b,0) + partial(b,1) + (bo + Wo @ bv).

v2 restructure vs v1 (104us -> target ~60us):
- fine-grained per-chunk input DMAs on three engine queues (sync: q-stream,
  scalar: k-stream, gpsimd: v-stream) so the projections chase the DMAs
  instead of waiting on monolithic packed loads (~14us startup stall)
- PE clock-ramp junk matmuls run on a memset tile (no DMA dependency)
- edge add moved off the PE (was I@edgeT matmuls) onto the DVE as a fused
  psum+sbuf->sbuf add feeding the exp
- v projection interleaved into head 0's score loop (chases the v DMA tail)
- PV matmuls software-pipelined one k-tile behind the score matmuls so the
  PE never waits on the Act-engine exp
- per-m-tile output stores issued as soon as each out-proj tile is copied
"""

import os
import sys

sys.path.insert(0, "/opt/trn_rl_repo")

import numpy as np

B, SEQ, DIN, DO = 4, 1024, 512, 512
NH_ALL, DK = 8, 64
NHC = 4            # heads per core
DH = NHC * DK      # 256 per-core projected dims
P = 128
CD = DIN // P      # 4 contraction chunks for projections
CH = DH // P       # 2 dh chunks
KT = SEQ // P      # 8 k-tiles
STR = 512          # q-stripe (matmul free dim)
NS = SEQ // STR    # 2 stripes
TVW = NHC * (DK + 1) + DK - 1  # 323: per-k-tile aux width (4x65 + 63 pad)

NJUNK0 = 7         # initial clock-ramp junk matmuls
NJUNK_BRIDGE = 16  # junk bridging the final normalize chain before out-proj

COMPUTE = os.environ.get("KERNEL_COMPUTE_DT", "fp16")  # fp16 | bf16 | fp32r

_nc = None


def _np_dt():
    import ml_dtypes

    return {
        "fp16": np.float16,
        "bf16": ml_dtypes.bfloat16,
        "fp32r": np.float32,
    }[COMPUTE]


def _build():
    global _nc
    if _nc is not None:
        return _nc
    import concourse.bacc as bacc
    import concourse.bass as bass
    import concourse.mybir as mybir
    import concourse.tile as tile

    f32 = mybir.dt.float32
    f32r = mybir.dt.float32r
    cdt = {
        "fp16": mybir.dt.float16,
        "bf16": mybir.dt.bfloat16,
        "fp32r": f32r,
    }[COMPUTE]
    Exp = mybir.ActivationFunctionType.Exp

    nc = bacc.Bacc("TRN2", target_bir_lowering=False, debug=False)

    wq_d = nc.dram_tensor("wq", (P, CD * DH), cdt, kind="ExternalInput")
    wk_d = nc.dram_tensor("wk", (P, CD * DH), cdt, kind="ExternalInput")
    wv_d = nc.dram_tensor("wv", (P, CD * DH), cdt, kind="ExternalInput")
    wo_d = nc.dram_tensor("wo", (P, CH * DO), cdt, kind="ExternalInput")
    xq_d = nc.dram_tensor("xq", (P, CD * SEQ), cdt, kind="ExternalInput")
    xk_d = nc.dram_tensor("xk", (P, CD * SEQ), cdt, kind="ExternalInput")
    xv_d = nc.dram_tensor("xv", (P, KT * CD * P), cdt, kind="ExternalInput")
    bqk = nc.dram_tensor("bqk", (2 * DH, 1), f32, kind="ExternalInput")
    edge = nc.dram_tensor("edge", (SEQ, SEQ), cdt, kind="ExternalInput")
    outp = nc.dram_tensor("outp", (SEQ, DO), cdt, kind="ExternalOutput")

    edge_r = edge.rearrange("(t p) n -> t p n", p=P)
    xq_r = xq_d.rearrange("p (c n) -> p c n", n=SEQ)
    xk_r = xk_d.rearrange("p (c n) -> p c n", n=SEQ)
    xv_r = xv_d.rearrange("p (t w) -> p t w", w=CD * P)
    out_r = outp.rearrange("(t p) n -> p t n", p=P)

    def sl(s):
        return slice(s * STR, (s + 1) * STR)

    with tile.TileContext(nc) as tc:
        with (
            tc.tile_pool(name="inp", bufs=1) as inp,
            tc.tile_pool(name="wts", bufs=1) as wts,
            tc.tile_pool(name="qkp", bufs=1) as qkp,
            tc.tile_pool(name="vhap", bufs=1) as vhap,
            tc.tile_pool(name="expp", bufs=6) as expp,
            tc.tile_pool(name="teip", bufs=3) as teip,
            tc.tile_pool(name="otp", bufs=1) as otp,
            tc.tile_pool(name="rrp", bufs=4) as rrp,
            tc.tile_pool(name="rbp", bufs=4) as rbp,
            tc.tile_pool(name="oalp", bufs=3) as oalp,
            tc.tile_pool(name="edgp", bufs=8) as edgp,
            tc.tile_pool(name="bigp", bufs=2, space=bass.MemorySpace.PSUM) as bigp,
            tc.tile_pool(name="pvp", bufs=3, space=bass.MemorySpace.PSUM) as pvp,
            tc.tile_pool(name="vpp", bufs=1, space=bass.MemorySpace.PSUM) as vpp,
        ):
            # ---------------- tiles ----------------
            tjk = wts.tile([P, STR], cdt, tag="tjk")
            twq = wts.tile([P, CD, DH], cdt, tag="twq")
            twk = wts.tile([P, CD, DH], cdt, tag="twk")
            twv = wts.tile([P, CD, DH], cdt, tag="twv")
            two = wts.tile([P, CH, DO], cdt, tag="two")
            tb4 = wts.tile([P, 4, 1], f32, tag="tb4")
            txq = inp.tile([P, CD, SEQ], cdt, tag="txq")
            txk = inp.tile([P, CD, SEQ], cdt, tag="txk")
            txv = inp.tile([P, KT, CD, P], cdt, tag="txv")
            tqh = qkp.tile([P, CH, SEQ], cdt, tag="tqh")
            khp = qkp.tile([P, NHC, SEQ], cdt, tag="khp")
            tvha = vhap.tile([P, KT, TVW], cdt, tag="tvha")
            tot = otp.tile([P, CH, SEQ], cdt, tag="tot")

            # ------- memsets: tjk on Pool (first op, gates junk); the rest on
            # DVE so the Pool queue can start issuing DMAs immediately -------
            nc.gpsimd.memset(tjk, 0.0)
            # zero the unused partition-halves of khp (even heads: parts
            # 64-127, odd heads: parts 0-63) so score matmuls see zero weights
            nc.vector.memset(khp[0:DK, 1::2, :], 0.0)
            nc.vector.memset(khp[DK:P, 0::2, :], 0.0)
            # vh-aug tail pad + per-head ones columns (denominator rows)
            nc.vector.memset(tvha[:, :, NHC * (DK + 1) : TVW], 0.0)
            nc.vector.memset(
                tvha[:, :, 0 : NHC * (DK + 1)].rearrange(
                    "p t (h w) -> p t h w", w=DK + 1
                )[:, :, :, DK : DK + 1],
                1.0,
            )

            # ------- input DMAs on sync + gpsimd ONLY (the scalar queue must
            # stay clear: exps queue behind anything issued on it).  Both
            # queues carry the critical q/k stream first, then v/edge. ------
            ed_tiles = [
                edgp.tile([P, SEQ], cdt, tag="edg", name=f"ed{kt}")
                for kt in range(KT)
            ]
            nc.sync.dma_start(out=tb4, in_=bqk.rearrange("(c p) o -> p c o", p=P))
            nc.sync.dma_start(out=twk, in_=wk_d.rearrange("p (c d) -> p c d", d=DH))
            nc.gpsimd.dma_start(
                out=twq, in_=wq_d.rearrange("p (c d) -> p c d", d=DH)
            )
            for cd in range(CD):
                nc.sync.dma_start(out=txk[:, cd], in_=xk_r[:, cd])
                nc.gpsimd.dma_start(out=txq[:, cd], in_=xq_r[:, cd])
            for kt in range(0, 4):
                nc.sync.dma_start(out=ed_tiles[kt], in_=edge_r[kt])
            nc.gpsimd.dma_start(
                out=twv, in_=wv_d.rearrange("p (c d) -> p c d", d=DH)
            )
            for st in range(KT):
                nc.gpsimd.dma_start(out=txv[:, st], in_=xv_r[:, st])
            nc.gpsimd.dma_start(
                out=two, in_=wo_d.rearrange("p (c d) -> p c d", d=DO)
            )
            for kt in range(4, KT):
                nc.gpsimd.dma_start(out=ed_tiles[kt], in_=edge_r[kt])

            # PE clock-ramp filler: junk matmuls on the memset tile keep the
            # p-state ramp going while DMAs land. Shares the vpp PSUM bank
            # (vproj runs much later); output never read.
            def junk(n):
                jt = vpp.tile([P, STR], f32, tag="vp")
                for _ in range(n):
                    nc.tensor.matmul(
                        jt[:], lhsT=tjk[:, 0:P], rhs=tjk[:], start=True, stop=True
                    )

            junk(NJUNK0)

            # ------- ch0 projections: k and q interleaved per DMA chunk ---
            # (ptq allocated first: its ring slot hosts the first stt and its
            # bias adds complete before ptk's)
            ptq = bigp.tile([P, SEQ], f32, tag="big")
            ptk = bigp.tile([P, SEQ], f32, tag="big")
            for cd in range(CD):
                for s in range(NS):
                    nc.tensor.matmul(
                        ptk[:, sl(s)],
                        lhsT=twk[:, cd, 0:P],
                        rhs=txk[:, cd, sl(s)],
                        start=(cd == 0),
                        stop=(cd == CD - 1),
                    )
                for s in range(NS):
                    nc.tensor.matmul(
                        ptq[:, sl(s)],
                        lhsT=twq[:, cd, 0:P],
                        rhs=txq[:, cd, sl(s)],
                        start=(cd == 0),
                        stop=(cd == CD - 1),
                    )
            # biases on DVE, ordered by gate: head 1 needs tqh-s0 + khp-odd
            # first; q-s1 before k-even so ptq (first stt slot) releases early
            nc.vector.tensor_scalar_add(
                out=tqh[:, 0, sl(0)], in0=ptq[:, sl(0)], scalar1=tb4[:, 0, :]
            )
            nc.vector.tensor_scalar_add(
                out=khp[DK:P, 1, :], in0=ptk[DK:P, :], scalar1=tb4[DK:P, 2, :]
            )
            nc.vector.tensor_scalar_add(
                out=tqh[:, 0, sl(1)], in0=ptq[:, sl(1)], scalar1=tb4[:, 0, :]
            )
            nc.vector.tensor_scalar_add(
                out=khp[0:DK, 0, :], in0=ptk[0:DK, :], scalar1=tb4[0:DK, 2, :]
            )

            def proj_ch1_mms():
                # pure-PE block between heads 1 and 0 (inputs already in
                # SBUF); the bias adds are deferred into head 0/2 DVE slack
                ptq2 = bigp.tile([P, SEQ], f32, tag="big")
                ptk2 = bigp.tile([P, SEQ], f32, tag="big")
                for cd in range(CD):
                    for s in range(NS):
                        nc.tensor.matmul(
                            ptq2[:, sl(s)],
                            lhsT=twq[:, cd, P : 2 * P],
                            rhs=txq[:, cd, sl(s)],
                            start=(cd == 0),
                            stop=(cd == CD - 1),
                        )
                    for s in range(NS):
                        nc.tensor.matmul(
                            ptk2[:, sl(s)],
                            lhsT=twk[:, cd, P : 2 * P],
                            rhs=txk[:, cd, sl(s)],
                            start=(cd == 0),
                            stop=(cd == CD - 1),
                        )
                return ptq2, ptk2

            # ---------------- attention per head ----------------
            # processing order: h1 (carries the v projection), h0 (edge adds
            # on DVE), h2, h3.  PV matmuls run one k-tile behind the scores.
            def head_body(h, with_vproj=False, extra_dve=None):
                # extra_dve: {kt: closure} — deferred DVE ops (ch1 proj
                # biases) emitted into this head's DVE slack after tile kt
                ch, off = h // 2, (h % 2) * DK
                edge_h = h == 0
                pv0 = pvp.tile([P, STR], f32, tag="pv")
                pv1 = pvp.tile([P, STR], f32, tag="pv")
                pvs = (pv0, pv1)

                def pv_mm(lte, lkt, stop):
                    for s in range(NS):
                        nc.tensor.matmul(
                            pvs[s][:],
                            lhsT=tvha[:, lkt, h * (DK + 1) : h * (DK + 1) + P],
                            rhs=lte[:, sl(s)],
                            start=(lkt == 0),
                            stop=stop,
                        )

                lag = None
                for kt in range(KT):
                    stt = bigp.tile([P, SEQ], f32, tag="big")
                    for s in range(NS):
                        nc.tensor.matmul(
                            stt[:, sl(s)],
                            lhsT=khp[:, h, kt * P : (kt + 1) * P],
                            rhs=tqh[:, ch, sl(s)],
                            start=True,
                            stop=True,
                        )
                    if with_vproj:
                        # v projection for k-tile kt, chasing the v DMA tail
                        vp = vpp.tile([P, STR], f32, tag="vp")
                        for cd in range(CD):
                            nc.tensor.matmul(
                                vp[:, 0:DH],
                                lhsT=txv[:, kt, cd, :],
                                rhs=twv[:, cd, :],
                                start=(cd == 0),
                                stop=(cd == CD - 1),
                            )
                        nc.vector.tensor_copy(
                            out=tvha[:, kt, 0 : NHC * (DK + 1)].rearrange(
                                "p (h w) -> p h w", w=DK + 1
                            )[:, :, 0:DK],
                            in_=vp[:, 0:DH].rearrange("p (h d) -> p h d", h=NHC),
                        )
                    te = expp.tile([P, SEQ], cdt, tag="expT")
                    if edge_h:
                        tei = teip.tile([P, SEQ], cdt, tag="tein")
                        nc.vector.tensor_add(
                            out=tei[:], in0=stt[:], in1=ed_tiles[kt][:]
                        )
                        nc.scalar.activation(out=te, in_=tei[:], func=Exp)
                    else:
                        nc.scalar.activation(out=te, in_=stt[:], func=Exp)
                    if extra_dve and kt in extra_dve:
                        extra_dve[kt]()
                    if lag is not None:
                        pv_mm(lag[0], lag[1], stop=False)
                    lag = (te, kt)
                pv_mm(lag[0], lag[1], stop=True)

                for s in range(NS):
                    rr = rrp.tile([1, STR], f32, tag="rr")
                    rs = rrp.tile([1, STR], f32, tag="rs")
                    nc.vector.tensor_copy(out=rs[:], in_=pvs[s][DK : DK + 1, :])
                    nc.vector.reciprocal_approx_fast(out=rr[:], in_=rs[:])
                    rb = rbp.tile([DK, STR], f32, tag="rb")
                    nc.gpsimd.partition_broadcast(rb[:], rr[:])
                    nc.vector.tensor_mul(
                        tot[off : off + DK, ch, sl(s)], pvs[s][0:DK, :], rb[:]
                    )

            head_body(1, with_vproj=True)
            ptq2, ptk2 = proj_ch1_mms()
            # ch1 biases: compact DVE block (must release the bigp slots
            # before head 0's stt tiles recycle them)
            nc.vector.tensor_scalar_add(
                out=tqh[:, 1, :], in0=ptq2[:], scalar1=tb4[:, 1, :]
            )
            nc.vector.tensor_scalar_add(
                out=khp[0:DK, 2, :], in0=ptk2[0:DK, :], scalar1=tb4[0:DK, 3, :]
            )
            nc.vector.tensor_scalar_add(
                out=khp[DK:P, 3, :], in0=ptk2[DK:P, :], scalar1=tb4[DK:P, 3, :]
            )
            head_body(0)
            head_body(2)
            head_body(3)
            junk(NJUNK_BRIDGE)

            # ---------------- output projection ----------------
            store_eng = [
                nc.sync, nc.gpsimd, nc.sync, nc.gpsimd,
                nc.sync, nc.gpsimd, nc.scalar, nc.scalar,
            ]
            for m in range(KT):
                po = bigp.tile([P, SEQ], f32, tag="big")
                for ci in range(CH):
                    nc.tensor.matmul(
                        po[:, 0:DO],
                        lhsT=tot[:, ci, m * P : (m + 1) * P],
                        rhs=two[:, ci, :],
                        start=(ci == 0),
                        stop=(ci == CH - 1),
                    )
                oal = oalp.tile([P, DO], cdt, tag="oall")
                nc.vector.tensor_copy(out=oal[:], in_=po[:, 0:DO])
                store_eng[m].dma_start(out=out_r[:, m], in_=oal[:])

    nc.compile()
    _nc = nc
    return nc


def _in_maps(q, k, v, edge_matrix, Wq, bq, Wk, bk, Wv, Wo):
    dt = _np_dt()
    zeros_edge = np.zeros((SEQ, SEQ), dt)
    edge_t = np.ascontiguousarray(edge_matrix.T).astype(dt)

    def re_cp(m):
        # [C*P, D] -> [P, C*D] (partition-major packing of "(c p) d -> p c d")
        cp, d = m.shape
        return np.ascontiguousarray(
            m.reshape(cp // P, P, d).transpose(1, 0, 2).reshape(P, -1)
        )

    def re_st(m):
        # [CD*P, KT*P] -> [P, KT*CD*P]: st-major packing for the v stream
        return np.ascontiguousarray(
            m.reshape(CD, P, KT, P).transpose(1, 2, 0, 3).reshape(P, -1)
        )

    xt = {}
    for b in range(B):
        xt[b] = (
            re_cp(np.ascontiguousarray(q[b].T).astype(dt)),
            re_cp(np.ascontiguousarray(k[b].T).astype(dt)),
            re_st(np.ascontiguousarray(v[b].T).astype(dt)),
        )
    maps = []
    for c in range(8):
        b, g = c // 2, c % 2
        is_edge = g == 0 and b < 2
        rows = slice(g * DH, (g + 1) * DH)
        wq_c = np.ascontiguousarray(Wq[rows].T) * np.float32(1.0 / 8.0)
        bq_c = (bq[rows] * np.float32(1.0 / 8.0)).copy()
        if is_edge:
            wq_c[:, 0:DK] = 0.0
            bq_c[0:DK] = 0.0
        maps.append(
            {
                "wq": re_cp(wq_c.astype(dt)),
                "wk": re_cp(np.ascontiguousarray(Wk[rows].T).astype(dt)),
                "wv": re_cp(np.ascontiguousarray(Wv[rows].T).astype(dt)),
                "wo": re_cp(np.ascontiguousarray(Wo[:, rows].T).astype(dt)),
                "xq": xt[b][0],
                "xk": xt[b][1],
                "xv": xt[b][2],
                "bqk": np.concatenate([bq_c, bk[rows]]).reshape(2 * DH, 1),
                "edge": edge_t if is_edge else zeros_edge,
            }
        )
    return maps


def _ensure_ntff_hook():
    """Register the axon NTFF profile hook if the image's antenv lacks it."""
    import contextlib
    import ctypes
    import types

    try:
        from antenv.axon_hooks import get_axon_ntff_profile_hook  # noqa: F401
        return
    except ImportError:
        pass

    so_path = "/opt/axon/libaxon_pjrt.so"
    try:
        lib = ctypes.CDLL(so_path)
    except OSError:
        return
    if not hasattr(lib, "axon_start_nrt_profile"):
        return
    lib.axon_start_nrt_profile.argtypes = [
        ctypes.POINTER(ctypes.c_int64),
        ctypes.c_size_t,
    ]
    lib.axon_start_nrt_profile.restype = ctypes.c_int64
    lib.axon_stop_nrt_profile.argtypes = [ctypes.c_char_p]
    lib.axon_stop_nrt_profile.restype = ctypes.c_int64

    @contextlib.contextmanager
    def _hook(output_dir, device_ids):
        import jax

        jax.devices()
        if device_ids:
            ids = (ctypes.c_int64 * len(device_ids))(*device_ids)
            rc = lib.axon_start_nrt_profile(ids, len(device_ids))
        else:
            rc = lib.axon_start_nrt_profile(None, 0)
        if rc != 0:
            raise RuntimeError(f"axon_start_nrt_profile rc={rc}")
        try:
            yield
        finally:
            n = lib.axon_stop_nrt_profile(str(output_dir).encode())
            if n < 0:
                raise RuntimeError(f"axon_stop_nrt_profile rc={n}")

    _state = {"hook": _hook}
    mod = types.ModuleType("antenv.axon_hooks")
    mod.get_axon_ntff_profile_hook = lambda: _state["hook"]
    mod.set_axon_ntff_profile_hook = lambda h: _state.__setitem__("hook", h)
    import antenv

    antenv.axon_hooks = mod
    sys.modules["antenv.axon_hooks"] = mod


def kernel(q, k, v, edge_matrix, Wq, bq, Wk, bk, Wv, bv, Wo, bo, _trace=False):
    from concourse.bass_utils import run_bass_kernel_spmd

    if _trace:
        _ensure_ntff_hook()

    q, k, v = (np.asarray(t, np.float32) for t in (q, k, v))
    edge_matrix = np.asarray(edge_matrix, np.float32)
    Wq, bq, Wk, bk, Wv, bv, Wo, bo = (
        np.asarray(t, np.float32) for t in (Wq, bq, Wk, bk, Wv, bv, Wo, bo)
    )

    nc = _build()
    maps = _in_maps(q, k, v, edge_matrix, Wq, bq, Wk, bk, Wv, Wo)
    res = run_bass_kernel_spmd(nc, maps, core_ids=list(range(8)), trace=_trace)

    bo_eff = bo + Wo @ bv
    out = np.empty((B, SEQ, DO), np.float32)
    for b in range(B):
        out[b] = res.results[2 * b]["outp"] + res.results[2 * b + 1]["outp"] + bo_eff
    if _trace:
        return out, res
    return out
